# revision 11
# baseline (speedup 1.0000x reference)
"""Trainium2 Bass kernel for nn_Attention_61168924229643.

4-head attention over 1024 tokens, dim_head=32, with the reference's quirks:
  - l2norm over the TOKEN axis (axis=1 of (B, HW, h, d)),
  - `attn - attn.argmax(-1)` before softmax, which is a per-row constant
    shift and cancels exactly inside jax.nn.softmax (which subtracts the
    row max internally). Logits are bounded (|S| < 1), so a raw
    exp/sum softmax reproduces the reference to ~2e-5.

Sharding: B=8 batch elements -> one per NeuronCore, no collectives.

Per-core layout is "transposed": tokens on the SBUF free axis, channels on
partitions. The token-axis l2norm becomes a free-axis reduction. Softmax
denominators ride along in the PV matmul as 32 extra ones-columns in the
stationary operand ([V_h | 1] per head), and the final projection uses
zero-padded per-head-pair w_out inputs so the denominator rows drop out
without any partition reshuffling.

The S^T matmuls use block-diagonal K weights (one head's rows, rest zero)
so all matmul operands sit at partition base 0 with K=128 (nonzero
tile_position row offsets crash the exec unit on this runtime, and
32-partition operands stream at reduced SBUF bandwidth).
"""

import numpy as np
import ml_dtypes
from contextlib import ExitStack

import concourse.bass as bass
import concourse.tile as tile
from concourse import bacc, mybir
from concourse.bass_utils import run_bass_kernel_spmd

FP32 = mybir.dt.float32
BF16 = mybir.dt.bfloat16

HW = 1024          # tokens per batch element (32*32)
C = 128            # channels
HEADS = 4
DH = 32            # dim per head
N_CORES = 8
NT = HW // 128     # 8 token tiles
LOG10 = float(np.log(10.0))


def build_kernel_body(ctx, tc, out_d, x_d, wqkv_d, ident_d, woa_d, wob_d, bias_d):
    nc = tc.nc
    Exp = mybir.ActivationFunctionType.Exp
    Ln = mybir.ActivationFunctionType.Ln
    Square = mybir.ActivationFunctionType.Square

    const = ctx.enter_context(tc.tile_pool(name="const", bufs=1))
    sb = ctx.enter_context(tc.tile_pool(name="sb", bufs=1))
    # PSUM: stp rotates 2x 4KB/partition tiles (2 banks each);
    # OA/OB accumulators 2 banks each. 4 + 2 + 2 = 8 banks.
    stp = ctx.enter_context(tc.tile_pool(name="stp", bufs=2, space="PSUM"))
    ops = ctx.enter_context(tc.tile_pool(name="ops", bufs=1, space="PSUM"))
    rps = ctx.enter_context(tc.tile_pool(name="rps", bufs=1, space="PSUM"))

    # ---- ACT table warm-up: touch Ln and Exp immediately so the table
    # loads overlap the input DMAs instead of stalling the S-pass later.
    warm = const.tile([128, 1], FP32, tag="warm")
    nc.vector.memset(warm[:], 1.0)
    warm2 = const.tile([128, 1], FP32, tag="warm2")
    nc.scalar.activation(warm2[:], warm[:], Ln)
    nc.scalar.activation(warm2[:], warm[:], Exp)

    # ---- constant inputs ----
    ident = const.tile([128, 128], FP32, tag="ident")
    nc.sync.dma_start(ident[:], ident_d[:])
    woa = const.tile([128, C], BF16, tag="woa")
    nc.sync.dma_start(woa[:], woa_d[:])
    wob = const.tile([128, C], BF16, tag="wob")
    nc.sync.dma_start(wob[:], wob_d[:])
    bias = const.tile([128, NT, C], FP32, tag="bias")
    nc.sync.dma_start(bias[:], bias_d[:])

    # ---- load x and weights ----
    xf = sb.tile([128, NT, C], FP32, tag="xf")  # partition = token%128
    nc.sync.dma_start(xf[:], x_d.rearrange("(t p) c -> p t c", p=128))
    wq = sb.tile([128, 3 * C], FP32, tag="wq")
    nc.gpsimd.dma_start(wq[:], wqkv_d[:])
    wqb = sb.tile([128, 3 * C], BF16, tag="wqb")
    nc.vector.tensor_copy(wqb[:], wq[:])

    # ---- background-initialized tiles ----
    # vb2[(j%128), t, h, 0:32] = V rows, [..., 32:64] stays 1.0 (denominator)
    vb2 = sb.tile([128, NT, HEADS, 2 * DH], BF16, tag="vb2")
    nc.vector.memset(vb2[:], 1.0)
    # rash/rbsh: reciprocal denominators aligned to O rows; background 1.0
    # keeps the unused rows finite so the full-width ops stay clean.
    rash = sb.tile([128, 1024], FP32, tag="rash")
    nc.vector.memset(rash[:], 1.0)
    rbsh = sb.tile([128, 1024], FP32, tag="rbsh")
    nc.vector.memset(rbsh[:], 1.0)

    # ---- X^T via PE transposes: [c, i] ----
    xt_ps = stp.tile([128, 1024], FP32, tag="st")
    for t in range(NT):
        nc.tensor.transpose(xt_ps[:, t * 128:(t + 1) * 128], xf[:, t, :],
                            ident[:], )
    xtb = sb.tile([128, NT, 128], BF16, tag="xtb")  # [c, (t, i_local)]
    nc.vector.tensor_copy(xtb[:], xt_ps[:].rearrange("p (t i) -> p t i", t=NT))
    xtb_flat = xtb[:].rearrange("p t i -> p (t i)")

    # ---- K^T first (its norm chain gates the S-pass) ----
    kt_ps = stp.tile([128, 1024], FP32, tag="st")
    for ih in range(2):
        nc.tensor.matmul(
            kt_ps[:, ih * 512:(ih + 1) * 512],
            lhsT=wqb[:, C:2 * C],
            rhs=xtb_flat[:, ih * 512:(ih + 1) * 512],
            start=True, stop=True,
        )
    nk2 = sb.tile([128, 1], FP32, tag="nk2")
    ksq_scr = sb.tile([128, 1024], FP32, tag="ksq_scr")
    nc.scalar.activation(ksq_scr[:], kt_ps[:], Square, accum_out=nk2[:])
    ktb = sb.tile([128, 1024], BF16, tag="ktb")
    nc.vector.tensor_copy(ktb[:], kt_ps[:])

    # ---- Q^T ----
    qt_ps = stp.tile([128, 1024], FP32, tag="st")
    for ih in range(2):
        nc.tensor.matmul(
            qt_ps[:, ih * 512:(ih + 1) * 512],
            lhsT=wqb[:, 0:C],
            rhs=xtb_flat[:, ih * 512:(ih + 1) * 512],
            start=True, stop=True,
        )
    nq2 = sb.tile([128, 1], FP32, tag="nq2")
    qsq_scr = sb.tile([128, 1024], FP32, tag="qsq_scr")
    nc.scalar.activation(qsq_scr[:], qt_ps[:], Square, accum_out=nq2[:])
    qtb = sb.tile([128, 1024], BF16, tag="qtb")
    nc.vector.tensor_copy(qtb[:], qt_ps[:])

    # scale s[(h,d)] = 10 / (||q_(h,d)|| * ||k_(h,d)||), eps-clipped like the
    # reference (clip(norm, 1e-12) == clip(norm^2, 1e-24) under sqrt).
    # s = exp(-0.5 * ln(nq2 * nk2) + ln(10)); Ln/Exp share one ACT table set.
    nq2c = sb.tile([128, 1], FP32, tag="nq2c")
    nc.vector.tensor_scalar_max(nq2c[:], nq2[:], 1e-24)
    nk2c = sb.tile([128, 1], FP32, tag="nk2c")
    nc.vector.tensor_scalar_max(nk2c[:], nk2[:], 1e-24)
    nn = sb.tile([128, 1], FP32, tag="nn")
    nc.vector.tensor_mul(nn[:], nq2c[:], nk2c[:])
    lnn = sb.tile([128, 1], FP32, tag="lnn")
    nc.scalar.activation(lnn[:], nn[:], Ln)
    log10_bias = sb.tile([128, 1], FP32, tag="log10_bias")
    nc.vector.memset(log10_bias[:], LOG10)
    sfac = sb.tile([128, 1], FP32, tag="sfac")
    nc.scalar.activation(sfac[:], lnn[:], Exp, bias=log10_bias[:], scale=-0.5)

    # Block-diagonal scaled K: ktbd[:, h, :] has rows 32h..32h+32 equal to
    # s * K^T (head h) and zeros elsewhere, so the S matmul runs with
    # K=128 full-width operands at partition base 0.
    ktbd = sb.tile([128, HEADS, 1024], BF16, tag="ktbd")
    nc.vector.memset(ktbd[:], 0.0)
    for h in range(4):
        nc.vector.tensor_scalar_mul(
            ktbd[32 * h:32 * (h + 1), h, :],
            ktb[32 * h:32 * (h + 1), :],
            sfac[32 * h:32 * (h + 1), :],
        )

    # ---- V in [token, f] orientation, scattered into vb2 ----
    v_ps = stp.tile([128, 1024], FP32, tag="st")
    for t in range(NT):
        nc.tensor.matmul(
            v_ps[:, t * 128:(t + 1) * 128],
            lhsT=xtb[:, t, :],
            rhs=wqb[:, 2 * C:3 * C],
            start=True, stop=True,
        )
    nc.vector.tensor_copy(
        vb2[:, :, :, 0:DH],
        v_ps[:].rearrange("p (t h d) -> p t h d", t=NT, h=HEADS),
    )

    # ---- attention ----
    # E^T[(jt, h)] = exp(S^T); S^T in bf16 PSUM so one matmul covers N=1024
    # and one ACT exp covers a head pair (FD=2048).
    eb = sb.tile([128, NT, HEADS, 1024], BF16, tag="eb")
    o_a = ops.tile([128, 1024], FP32, tag="oa")  # heads 0/1: [O0,r0,O1,r1]
    o_b = rps.tile([128, 1024], FP32, tag="ob")  # heads 2/3

    for jt in range(NT):
        for h in range(4):
            st = stp.tile([128, 1024], FP32, tag="st")
            for ih in range(2):
                nc.tensor.matmul(
                    st[:, ih * 512:(ih + 1) * 512],
                    lhsT=ktbd[:, h, jt * 128:(jt + 1) * 128],
                    rhs=qtb[:, ih * 512:(ih + 1) * 512],
                    start=True, stop=True,
                )
            nc.scalar.activation(eb[:, jt, h, :], st[:], Exp)
        for ih in range(2):
            for h in range(4):
                dst = o_a if h < 2 else o_b
                nc.tensor.matmul(
                    dst[64 * (h % 2):64 * (h % 2) + 64,
                        ih * 512:(ih + 1) * 512],
                    lhsT=vb2[:, jt, h, :],
                    rhs=eb[:, jt, h, ih * 512:(ih + 1) * 512],
                    start=(jt == 0), stop=(jt == NT - 1),
                    tile_position=(0, 64 * (h % 2)),
                    skip_group_check=True,
                )

    # ---- normalize + output projection ----
    # o_a rows: [O_0 (0:32), r_0 (32:64), O_1 (64:96), r_1 (96:128)].
    # rash rows 0:32 <- 1/r_0, rows 64:96 <- 1/r_1 (rest stays 1.0), so one
    # full-width multiply normalizes the O rows in place; the junk rows
    # (r * 1/junk) are killed by the zero rows of woa/wob.
    ra = sb.tile([128, 1024], FP32, tag="ra")
    nc.vector.reciprocal_approx_fast(ra[:], o_a[:])
    nc.sync.dma_start(rash[0:32, :], ra[32:64, :])
    nc.sync.dma_start(rash[64:96, :], ra[96:128, :])
    rb = sb.tile([128, 1024], FP32, tag="rb")
    nc.vector.reciprocal_approx_fast(rb[:], o_b[:])
    nc.gpsimd.dma_start(rbsh[0:32, :], rb[32:64, :])
    nc.gpsimd.dma_start(rbsh[64:96, :], rb[96:128, :])
    stack_a = sb.tile([128, 1024], BF16, tag="stack_a")
    nc.vector.tensor_mul(stack_a[:], o_a[:], rash[:])
    stack_b = sb.tile([128, 1024], BF16, tag="stack_b")
    nc.vector.tensor_mul(stack_b[:], o_b[:], rbsh[:])

    y_ps = stp.tile([128, 1024], FP32, tag="st")
    for it in range(NT):
        nc.tensor.matmul(
            y_ps[:, it * 128:(it + 1) * 128],
            lhsT=stack_a[:, it * 128:(it + 1) * 128],
            rhs=woa[:],
            start=True, stop=False,
            skip_group_check=True,
        )
        nc.tensor.matmul(
            y_ps[:, it * 128:(it + 1) * 128],
            lhsT=stack_b[:, it * 128:(it + 1) * 128],
            rhs=wob[:],
            start=False, stop=True,
            skip_group_check=True,
        )
    yout = sb.tile([128, NT, C], FP32, tag="yout")
    nc.vector.tensor_add(
        yout[:], y_ps[:].rearrange("p (t c) -> p t c", t=NT), bias[:])
    nc.sync.dma_start(out_d.rearrange("(t p) c -> p t c", p=128), yout[:])


def build_nc():
    nc = bacc.Bacc("TRN2", target_bir_lowering=False, debug=False,
                   num_devices=N_CORES)
    x_d = nc.dram_tensor("x", [HW, C], FP32, kind="ExternalInput").ap()
    wqkv_d = nc.dram_tensor("w_qkv", [C, 3 * C], FP32, kind="ExternalInput").ap()
    ident_d = nc.dram_tensor("ident", [128, 128], FP32, kind="ExternalInput").ap()
    woa_d = nc.dram_tensor("woa", [128, C], BF16, kind="ExternalInput").ap()
    wob_d = nc.dram_tensor("wob", [128, C], BF16, kind="ExternalInput").ap()
    bias_d = nc.dram_tensor("bias", [128, NT, C], FP32, kind="ExternalInput").ap()
    out_d = nc.dram_tensor("out", [HW, C], FP32, kind="ExternalOutput").ap()
    with tile.TileContext(nc) as tc:
        with ExitStack() as ctx:
            build_kernel_body(ctx, tc, out_d, x_d, wqkv_d, ident_d,
                              woa_d, wob_d, bias_d)
    nc.compile()
    return nc


_CACHED_NC = None


def get_nc():
    global _CACHED_NC
    if _CACHED_NC is None:
        _CACHED_NC = build_nc()
    return _CACHED_NC


def make_in_maps(x, w_qkv, w_out, b_out):
    x = np.ascontiguousarray(np.asarray(x, dtype=np.float32)).reshape(N_CORES, HW, C)
    w_qkv = np.ascontiguousarray(np.asarray(w_qkv, dtype=np.float32))
    w_out = np.asarray(w_out, dtype=np.float32)
    b_out = np.asarray(b_out, dtype=np.float32).reshape(C)

    ident = np.eye(128, dtype=np.float32)
    # woa: rows [w_out[0:32]; 0; w_out[32:64]; 0]  (heads 0, 1)
    # wob: rows [w_out[64:96]; 0; w_out[96:128]; 0]  (heads 2, 3)
    woa = np.zeros((128, C), dtype=np.float32)
    wob = np.zeros((128, C), dtype=np.float32)
    woa[0:32] = w_out[0:32]
    woa[64:96] = w_out[32:64]
    wob[0:32] = w_out[64:96]
    wob[64:96] = w_out[96:128]
    woa = woa.astype(ml_dtypes.bfloat16)
    wob = wob.astype(ml_dtypes.bfloat16)
    bias = np.ascontiguousarray(
        np.broadcast_to(b_out[None, None, :], (128, NT, C)).astype(np.float32))
    return [
        {"x": x[i], "w_qkv": w_qkv, "ident": ident, "woa": woa, "wob": wob,
         "bias": bias}
        for i in range(N_CORES)
    ]


def kernel(x, w_qkv, w_out, b_out, _trace=False, _trace_kwargs=None):
    nc = get_nc()
    in_maps = make_in_maps(x, w_qkv, w_out, b_out)
    res = run_bass_kernel_spmd(
        nc, in_maps, core_ids=list(range(N_CORES)),
        trace=_trace, **(_trace_kwargs or {}),
    )
    out = np.stack([np.asarray(res.results[i]["out"]) for i in range(N_CORES)])
    out = out.reshape(8, 32, 32, 128).astype(np.float32)
    if _trace:
        kernel.last_result = res
    return out
